# revision 11
# baseline (speedup 1.0000x reference)
"""AdaptiveLogSoftmaxWithLoss (with ignore_index) on 8 TRN2 NeuronCores.

Strategy (data-parallel over the token axis, with host-side class-sorted
row partitioning):
  - Rows are bucketed by target class band (cluster1 / cluster0 /
    head-or-ignored), padded to fixed per-core section sizes, and sharded
    so every core gets the same per-section row counts:
        [c1: 448][zero-pad: 64][c0: 128][head-only: 128]  = 768 rows/core.
  - The expensive per-band partition sums then run only on the rows that
    need them: head logsumexp over all 768 rows, cluster0 (8000 classes)
    over 128 rows, cluster1 (40257 classes) over 448 rows.
  - Weights are replicated (bf16, pre-transposed on host).
  - Per-row target logits come from an indirect-DMA gather of the target's
    weight row + a multiply-reduce dot, so full logits are never gathered.
  - Per-core masked loss numerator + valid count are returned and combined
    on the host (the 8-way sum + divide).

kernel(**inputs) takes the FULL unsharded inputs and returns the scalar loss.
"""

import sys
import types

import ml_dtypes
import numpy as np

# ---------------------------------------------------------------- constants
CORES = 8
N_ROWS = 4096
D = 1024
KT = D // 128                  # k tiles = 8
HEAD = 2002                    # head classes (2000 + 2 cluster slots)
C0_LOW, C0_HIGH, NCLS = 2000, 10000, 50257
C0 = C0_HIGH - C0_LOW          # 8000
C1 = NCLS - C0_HIGH            # 40257
H0, H1 = 256, 64
IGNORE = -1

# class-sorted layout (per core)
S_C1, S_PAD, S_C0, S_HD = 448, 64, 128, 128
RPC = S_C1 + S_PAD + S_C0 + S_HD          # 768 rows per core
TILES = RPC // 128                        # 6 row tiles per core
CAP_C1, CAP_C0, CAP_HD = S_C1 * CORES, S_C0 * CORES, S_HD * CORES

HEAD_TILES = [(t, 128) for t in range(TILES)]
C0_TILES = [(4, 128)]
C1_TILES = [(0, 128), (1, 128), (2, 128), (3, 64)]

BF16 = ml_dtypes.bfloat16

_CACHE = {}


def _install_profile_hook():
    """Register the axon NTFF profile hook (missing from the image's antenv)
    so run_bass_kernel_spmd(trace=True) can return exec_time_ns."""
    if "antenv.axon_hooks" in sys.modules:
        return
    try:
        mod = types.ModuleType("antenv.axon_hooks")
        state = {"hook": None}
        mod.set_axon_ntff_profile_hook = lambda h: state.update(hook=h)
        mod.get_axon_ntff_profile_hook = lambda: state["hook"]
        sys.modules["antenv.axon_hooks"] = mod
        import antenv

        antenv.axon_hooks = mod
        from trn_agent_boot.trn_boot import _ntff_profile_via_ctypes

        mod.set_axon_ntff_profile_hook(
            _ntff_profile_via_ctypes("/opt/axon/libaxon_pjrt.so")
        )
        from concourse import bass_utils

        bass_utils.upload_artifacts = lambda tmpdir: tmpdir
    except Exception:
        pass


def _enable_ldw_opt():
    """Flip walrus --enable-ldw-opt to true: consecutive matmuls that reuse
    the same stationary operand then skip the redundant LDWEIGHTS."""
    from concourse import bass_utils

    if getattr(bass_utils, "_ldw_patched", False):
        return
    orig = bass_utils.run_command

    def patched(cmd, **kw):
        cmd = [
            "--enable-ldw-opt=true" if c == "--enable-ldw-opt=false" else c
            for c in cmd
        ]
        return orig(cmd, **kw)

    bass_utils.run_command = patched
    bass_utils._ldw_patched = True


def _chunks(total, size):
    return [(a, min(a + size, total)) for a in range(0, total, size)]


def build_nc(use_collective=False, act_span=2048):
    from concourse import bacc, bass, mybir, tile

    fp32 = mybir.dt.float32
    bf16 = mybir.dt.bfloat16
    i32 = mybir.dt.int32

    nc = bacc.Bacc("TRN2", target_bir_lowering=False, debug=False, num_devices=CORES)
    # Keep matmuls un-split (no standalone InstLdweights) so walrus'
    # ldw-opt can elide redundant weight loads for consecutive matmuls
    # that share the same stationary operand.
    nc.move_matmul_waits_to_ldweights = lambda: None

    # ------------------------------------------------------------ parameters
    def param(name, shape, dt):
        return nc.declare_dram_parameter(name, list(shape), dt, isOutput=False)

    xt_d = param("xt", [KT, 128, RPC], bf16)           # x shard, transposed
    xr_d = param("xr", [TILES, 128, D], bf16)          # x shard, row-major
    gidx_d = param("gidx", [TILES, 128, 1], i32)       # head gather row idx
    r0idx_d = param("r0idx", [TILES, 128, 1], i32)
    r1idx_d = param("r1idx", [TILES, 128, 1], i32)
    valid_d = param("validm", [128, TILES], fp32)      # [p, t] masks
    c0m_d = param("c0m", [128, TILES], fp32)
    c1m_d = param("c1m", [128, TILES], fp32)
    hwt_d = param("hwt", [KT, 128, HEAD], bf16)        # head_w.T k-tiles
    hwr_d = param("hwr", [HEAD, D], bf16)              # head_w row-major
    p0t_d = param("p0t", [KT, 128, H0], bf16)
    p1t_d = param("p1t", [KT, 128, H1], bf16)
    o0t_d = param("o0t", [2, 128, C0], bf16)           # out0.T k-tiles
    o0r_d = param("o0r", [C0, H0], bf16)
    o1t_d = param("o1t", [128, 20 * 1024], bf16)       # out1.T quadrant-packed
    o1r_d = param("o1r", [C1, H1], bf16)

    out_shape = [1, 1] if use_collective else [1, 2]
    out_d = nc.declare_dram_parameter("out", out_shape, fp32, isOutput=True)

    head_slices = _chunks(HEAD, 512)
    c0_chunks = _chunks(C0, 2048)       # 4 chunks
    c1_chunks = _chunks(C1, 2048)       # 20 chunks

    def n_parts(total):
        return sum(len(_chunks(b - a, act_span)) for a, b in _chunks(total, 2048))

    nh, nc0p, nc1p = len(_chunks(HEAD, 512)), n_parts(C0), n_parts(C1)

    with tile.TileContext(nc) as tc:
        with (
            tc.tile_pool(name="res", bufs=1) as res,       # resident SBUF
            tc.tile_pool(name="wstream", bufs=2) as wst,   # streamed weights
            tc.tile_pool(name="scr", bufs=1) as scr,       # ACT exp scratch
            tc.tile_pool(name="dram", bufs=1, space="DRAM") as dram,
        ):
            # -------------------------------------------- resident loads
            # Order matters: it is both the DMA queue order and (roughly) the
            # per-engine program order. xt + head slices first so the head
            # matmul pipeline starts as soon as (k=0, s=0) land.
            xt_sb = []
            for k in range(KT):
                t_ = res.tile([128, RPC], bf16, tag=f"xt{k}", name=f"xt{k}")
                nc.sync.dma_start(out=t_[:, :], in_=xt_d.ap()[k])
                xt_sb.append(t_)
            p0t_sb, p1t_sb = [], []
            for k in range(KT):
                a = res.tile([128, H0], bf16, tag=f"p0t{k}", name=f"p0t{k}")
                nc.sync.dma_start(out=a[:, :], in_=p0t_d.ap()[k])
                p0t_sb.append(a)
                b = res.tile([128, H1], bf16, tag=f"p1t{k}", name=f"p1t{k}")
                nc.sync.dma_start(out=b[:, :], in_=p1t_d.ap()[k])
                p1t_sb.append(b)
            # prefetch the first cluster-1 weight chunks so the c1 pipeline
            # starts before the bulk resident loads finish
            N_W1_PREF = 3
            w1_tiles = {}
            for ci in range(N_W1_PREF):
                w = wst.tile([128, 1024], bf16, tag="w1", name="w1", bufs=N_W1_PREF + 2)
                nc.sync.dma_start(
                    out=w[:, :], in_=o1t_d.ap()[:, 1024 * ci : 1024 * (ci + 1)]
                )
                w1_tiles[ci] = w
            hwt_sb = {}
            for k in range(KT):
                for si, (s0, s1) in enumerate(head_slices):
                    t_ = res.tile(
                        [128, s1 - s0], bf16, tag=f"hwt{k}_{si}", name=f"hwt{k}_{si}"
                    )
                    nc.sync.dma_start(out=t_[:, :], in_=hwt_d.ap()[k, :, s0:s1])
                    hwt_sb[(k, si)] = t_
            xr_sb = []
            for t in range(TILES):
                t_ = res.tile([128, D], bf16, tag=f"xr{t}", name=f"xr{t}")
                nc.sync.dma_start(out=t_[:, :], in_=xr_d.ap()[t])
                xr_sb.append(t_)
            idx_sb = {}
            for name, d in (("g", gidx_d), ("r0", r0idx_d), ("r1", r1idx_d)):
                for t in range(TILES):
                    t_ = res.tile(
                        [128, 1], i32, tag=f"idx{name}{t}", name=f"idx{name}{t}"
                    )
                    nc.sync.dma_start(out=t_[:, :], in_=d.ap()[t])
                    idx_sb[(name, t)] = t_
            valid_sb = res.tile([128, TILES], fp32, tag="valid", name="valid")
            c0m_sb = res.tile([128, TILES], fp32, tag="c0m", name="c0m")
            c1m_sb = res.tile([128, TILES], fp32, tag="c1m", name="c1m")
            nc.sync.dma_start(out=valid_sb[:, :], in_=valid_d.ap()[:, :])
            nc.sync.dma_start(out=c0m_sb[:, :], in_=c0m_d.ap()[:, :])
            nc.sync.dma_start(out=c1m_sb[:, :], in_=c1m_d.ap()[:, :])

            # -------------------------------------------- indirect gathers
            hwg_sb, o0g_sb, o1g_sb = [], [], []
            for t in range(TILES):
                g = res.tile([128, D], bf16, tag=f"hwg{t}", name=f"hwg{t}")
                nc.gpsimd.indirect_dma_start(
                    out=g[:, :], out_offset=None, in_=hwr_d.ap()[:, :],
                    in_offset=bass.IndirectOffsetOnAxis(
                        ap=idx_sb[("g", t)][:, :1], axis=0
                    ),
                )
                hwg_sb.append(g)
                g0 = res.tile([128, H0], bf16, tag=f"o0g{t}", name=f"o0g{t}")
                nc.gpsimd.indirect_dma_start(
                    out=g0[:, :], out_offset=None, in_=o0r_d.ap()[:, :],
                    in_offset=bass.IndirectOffsetOnAxis(
                        ap=idx_sb[("r0", t)][:, :1], axis=0
                    ),
                )
                o0g_sb.append(g0)
                g1 = res.tile([128, H1], bf16, tag=f"o1g{t}", name=f"o1g{t}")
                nc.gpsimd.indirect_dma_start(
                    out=g1[:, :], out_offset=None, in_=o1r_d.ap()[:, :],
                    in_offset=bass.IndirectOffsetOnAxis(
                        ap=idx_sb[("r1", t)][:, :1], axis=0
                    ),
                )
                o1g_sb.append(g1)

            # -------------------------------------------- SBUF result tiles
            p0T_sb = res.tile([128, S_C0], bf16, tag="p0T_a", name="p0T_a")
            p0T_sb2 = res.tile([128, S_C0], bf16, tag="p0T_b", name="p0T_b")
            p1T_sb = res.tile([128, S_C1 + S_PAD], bf16, tag="p1T", name="p1T")
            p0r_sb = {}
            p1r_sb = {}
            for t, m in C0_TILES:
                p0r_sb[t] = res.tile([128, H0], bf16, tag=f"p0r{t}", name=f"p0r{t}")
            for t, m in C1_TILES:
                p1r_sb[t] = res.tile([128, H1], bf16, tag=f"p1r{t}", name=f"p1r{t}")
            hd_sb = res.tile([128, TILES], fp32, tag="hd", name="hd")
            cd0_sb = res.tile([128, TILES], fp32, tag="cd0", name="cd0")
            cd1_sb = res.tile([128, TILES], fp32, tag="cd1", name="cd1")
            F_sb = res.tile([128, 3 * TILES], fp32, tag="F", name="F")
            zh_parts = res.tile([128, TILES * nh], fp32, tag="zhp", name="zhp")
            zc0_parts = res.tile([128, TILES * nc0p], fp32, tag="zc0p", name="zc0p")
            zc1_parts = res.tile([128, TILES * nc1p], fp32, tag="zc1p", name="zc1p")
            exp_scr = scr.tile([128, 2048], bf16, tag="exp", name="exp")

            # sections only write their own tiles/partitions; everything else
            # must be a harmless finite value (exp-sum 1.0 -> Ln finite).
            nc.gpsimd.memset(zc0_parts[:, :], 1.0)
            nc.gpsimd.memset(zc1_parts[:, :], 1.0)
            nc.gpsimd.memset(zh_parts[:, :], 1.0)
            nc.gpsimd.memset(cd0_sb[:, :], 0.0)
            nc.gpsimd.memset(cd1_sb[:, :], 0.0)

            def act_exp(ps, m, cw, parts, t, npart, base):
                """exp + accumulate psum[:m, :cw] in act_span pieces."""
                for j, (a0, a1) in enumerate(_chunks(cw, act_span)):
                    col = t * npart + base + j
                    nc.scalar.activation(
                        exp_scr[:m, a0:a1],
                        ps[:m, a0:a1],
                        mybir.ActivationFunctionType.Exp,
                        accum_out=parts[:m, col : col + 1],
                    )

            with tc.tile_pool(name="big", bufs=2, space="PSUM") as bigp:

                def big_psum():
                    return bigp.tile([128, 2048], fp32, tag="big", name="big")

                # ---------------- projections first (warms PE, unblocks c1)
                # p1T: the c1 section rows (tiles 0..3 incl pad)
                ps = big_psum()
                for k in range(KT):
                    nc.tensor.matmul(
                        ps[:64, : S_C1 + S_PAD],
                        lhsT=p1t_sb[k][:, :],
                        rhs=xt_sb[k][:, : S_C1 + S_PAD],
                        start=(k == 0),
                        stop=(k == KT - 1),
                    )
                nc.vector.tensor_copy(p1T_sb[0:64, :], ps[:64, : S_C1 + S_PAD])
                nc.vector.tensor_copy(p1T_sb[64:128, :], ps[:64, : S_C1 + S_PAD])
                # p0T: only the c0 section rows (tile 4)
                c0_t0 = C0_TILES[0][0]
                for mi, dst in ((0, p0T_sb), (1, p0T_sb2)):
                    ps = big_psum()
                    for k in range(KT):
                        nc.tensor.matmul(
                            ps[:, : S_C0],
                            lhsT=p0t_sb[k][:, 128 * mi : 128 * (mi + 1)],
                            rhs=xt_sb[k][:, 128 * c0_t0 : 128 * c0_t0 + S_C0],
                            start=(k == 0),
                            stop=(k == KT - 1),
                        )
                    nc.vector.tensor_copy(dst[:, :], ps[:, : S_C0])
                # row-major projections for the dot products
                for t, m in C0_TILES:
                    ps = big_psum()
                    for k in range(KT):
                        nc.tensor.matmul(
                            ps[:m, :H0],
                            lhsT=xt_sb[k][:, 128 * t : 128 * t + m],
                            rhs=p0t_sb[k][:, :],
                            start=(k == 0),
                            stop=(k == KT - 1),
                        )
                    nc.vector.tensor_copy(p0r_sb[t][:m, :], ps[:m, :H0])
                for t, m in C1_TILES:
                    ps = big_psum()
                    for k in range(KT):
                        nc.tensor.matmul(
                            ps[:m, :H1],
                            lhsT=xt_sb[k][:, 128 * t : 128 * t + m],
                            rhs=p1t_sb[k][:, :],
                            start=(k == 0),
                            stop=(k == KT - 1),
                        )
                    nc.vector.tensor_copy(p1r_sb[t][:m, :], ps[:m, :H1])

                # ---------------- merged schedule: c1 (chunk, tile) units
                # paced with fine-grained head/c0 fillers. c1 is ACT-heavy /
                # PE-light; head is the reverse. Head fillers are single
                # 512-col slices so no unit holds a PSUM slot for long.
                def emit_head_slice(t, m, si):
                    s0, s1 = head_slices[si]
                    ps = big_psum()
                    for k in range(KT):
                        nc.tensor.matmul(
                            ps[:m, : s1 - s0],
                            lhsT=xt_sb[k][:, 128 * t : 128 * t + m],
                            rhs=hwt_sb[(k, si)][:, :],
                            start=(k == 0),
                            stop=(k == KT - 1),
                        )
                    nc.scalar.activation(
                        exp_scr[:m, : s1 - s0],
                        ps[:m, : s1 - s0],
                        mybir.ActivationFunctionType.Exp,
                        accum_out=zh_parts[:m, t * nh + si : t * nh + si + 1],
                    )

                def emit_c0_chunk(ci):
                    c0, c1_ = c0_chunks[ci]
                    w = [
                        wst.tile([128, 2048], bf16, tag="w0a", name="w0a"),
                        wst.tile([128, 2048], bf16, tag="w0b", name="w0b"),
                    ]
                    cw = c1_ - c0
                    for k in range(2):
                        nc.sync.dma_start(
                            out=w[k][:, :cw], in_=o0t_d.ap()[k, :, c0:c1_]
                        )
                    pbase = sum(
                        len(_chunks(b - a, act_span)) for a, b in c0_chunks[:ci]
                    )
                    for t, m in C0_TILES:
                        ps = big_psum()
                        p0T = (p0T_sb, p0T_sb2)
                        for k in range(2):
                            for s0, s1 in _chunks(cw, 512):
                                nc.tensor.matmul(
                                    ps[:m, s0:s1],
                                    lhsT=p0T[k][:, :m],
                                    rhs=w[k][:, s0:s1],
                                    start=(k == 0),
                                    stop=(k == 1),
                                )
                        act_exp(ps, m, cw, zc0_parts, t, nc0p, pbase)

                fillers = [
                    ("head", t, si)
                    for t, m in HEAD_TILES
                    for si in range(len(head_slices))
                ] + [("c0", ci, 0) for ci in range(len(c0_chunks))]
                n_units = len(c1_chunks) * len(C1_TILES)
                fper = len(fillers) / n_units
                fcredit = 0.0
                fi = 0
                for ci in range(len(c1_chunks)):
                    c0, c1_ = c1_chunks[ci]
                    cw = c1_ - c0          # real classes this chunk (<= 2048)
                    if ci in w1_tiles:
                        w = w1_tiles.pop(ci)
                    else:
                        w = wst.tile(
                            [128, 1024], bf16, tag="w1", name="w1", bufs=N_W1_PREF + 2
                        )
                        nc.sync.dma_start(
                            out=w[:, :], in_=o1t_d.ap()[:, 1024 * ci : 1024 * (ci + 1)]
                        )
                    pbase = sum(
                        len(_chunks(b - a, act_span)) for a, b in c1_chunks[:ci]
                    )
                    for t, m in C1_TILES:
                        ps = big_psum()
                        # quadrant-packed: top half of w = classes [0,1024),
                        # bottom half = classes [1024,2048) of this chunk.
                        # Alternate quadrants so LDWEIGHTS overlaps matmuls.
                        mms = []
                        for s0, s1 in _chunks(min(cw, 1024), 512):
                            mms.append((0, s0, s1))
                        if cw > 1024:
                            for s0, s1 in _chunks(cw - 1024, 512):
                                mms.append((1, s0, s1))
                        order = [x for pair in zip(mms[:2], mms[2:]) for x in pair]
                        if len(order) < len(mms):
                            order += mms[len(order):]
                        for h, s0, s1 in order:
                            nc.tensor.matmul(
                                ps[:m, 1024 * h + s0 : 1024 * h + s1],
                                lhsT=p1T_sb[64 * h : 64 * h + 64, 128 * t : 128 * t + m],
                                rhs=w[64 * h : 64 * h + 64, s0:s1],
                                start=True,
                                stop=True,
                            )
                        act_exp(ps, m, cw, zc1_parts, t, nc1p, pbase)
                        fcredit += fper
                        while fcredit >= 1.0 and fi < len(fillers):
                            kind, a, b = fillers[fi]
                            fi += 1
                            fcredit -= 1.0
                            if kind == "head":
                                emit_head_slice(a, 128, b)
                            else:
                                emit_c0_chunk(a)
                while fi < len(fillers):
                    kind, a, b = fillers[fi]
                    fi += 1
                    if kind == "head":
                        emit_head_slice(a, 128, b)
                    else:
                        emit_c0_chunk(a)

                # ---------------- dots (DVE, off critical path)
                dots_scr = scr.tile([128, D], bf16, tag="dots", name="dots")
                for t in range(TILES):
                    nc.vector.tensor_tensor(
                        dots_scr[:, :D], xr_sb[t][:, :], hwg_sb[t][:, :],
                        mybir.AluOpType.mult,
                    )
                    nc.vector.tensor_reduce(
                        out=hd_sb[:, t : t + 1], in_=dots_scr[:, :D],
                        axis=mybir.AxisListType.X, op=mybir.AluOpType.add,
                    )
                for t, m in C0_TILES:
                    nc.vector.tensor_tensor(
                        dots_scr[:m, :H0], p0r_sb[t][:m, :], o0g_sb[t][:m, :],
                        mybir.AluOpType.mult,
                    )
                    nc.vector.tensor_reduce(
                        out=cd0_sb[:m, t : t + 1], in_=dots_scr[:m, :H0],
                        axis=mybir.AxisListType.X, op=mybir.AluOpType.add,
                    )
                for t, m in C1_TILES:
                    nc.vector.tensor_tensor(
                        dots_scr[:m, :H1], p1r_sb[t][:m, :], o1g_sb[t][:m, :],
                        mybir.AluOpType.mult,
                    )
                    nc.vector.tensor_reduce(
                        out=cd1_sb[:m, t : t + 1], in_=dots_scr[:m, :H1],
                        axis=mybir.AxisListType.X, op=mybir.AluOpType.add,
                    )

            # -------------------------------------------- per-row loss
            for t in range(TILES):
                for sec_i, (parts, npart) in enumerate(
                    ((zh_parts, nh), (zc0_parts, nc0p), (zc1_parts, nc1p))
                ):
                    nc.vector.tensor_reduce(
                        out=F_sb[:, sec_i * TILES + t : sec_i * TILES + t + 1],
                        in_=parts[:, t * npart : (t + 1) * npart],
                        axis=mybir.AxisListType.X,
                        op=mybir.AluOpType.add,
                    )
            L_sb = res.tile([128, 3 * TILES], fp32, tag="L", name="L")
            nc.scalar.activation(
                L_sb[:, :], F_sb[:, :], mybir.ActivationFunctionType.Ln
            )
            T1 = res.tile([128, TILES], fp32, tag="T1", name="T1")
            T2 = res.tile([128, TILES], fp32, tag="T2", name="T2")
            R_sb = res.tile([128, 2 * TILES], bf16, tag="R", name="R")
            sub, mult, add = (
                mybir.AluOpType.subtract,
                mybir.AluOpType.mult,
                mybir.AluOpType.add,
            )
            nc.vector.tensor_tensor(T1[:, :], hd_sb[:, :], L_sb[:, 0:TILES], sub)
            nc.vector.tensor_tensor(
                T2[:, :], cd0_sb[:, :], L_sb[:, TILES : 2 * TILES], sub
            )
            nc.vector.tensor_tensor(T2[:, :], T2[:, :], c0m_sb[:, :], mult)
            nc.vector.tensor_tensor(T1[:, :], T1[:, :], T2[:, :], add)
            nc.vector.tensor_tensor(
                T2[:, :], cd1_sb[:, :], L_sb[:, 2 * TILES : 3 * TILES], sub
            )
            nc.vector.tensor_tensor(T2[:, :], T2[:, :], c1m_sb[:, :], mult)
            nc.vector.tensor_tensor(T1[:, :], T1[:, :], T2[:, :], add)
            nc.vector.tensor_tensor(R_sb[:, 0:TILES], T1[:, :], valid_sb[:, :], mult)
            nc.vector.tensor_copy(R_sb[:, TILES : 2 * TILES], valid_sb[:, :])

            # -------------------------------------------- reduce across rows
            ones_sb = res.tile([128, 1], bf16, tag="ones", name="ones")
            nc.gpsimd.memset(ones_sb[:, :], 1.0)
            S_sb = res.tile([1, 2 * TILES], fp32, tag="S", name="S")
            cc_in = res.tile([1, 16], fp32, tag="ccin", name="ccin")
            nc.gpsimd.memset(cc_in[:, :], 0.0)
            with tc.tile_pool(name="sm", bufs=1, space="PSUM") as smp:
                ps = smp.tile([1, 2 * TILES], fp32, tag="sm", name="sm")
                nc.tensor.matmul(
                    ps[:, :], lhsT=ones_sb[:, :], rhs=R_sb[:, :], start=True, stop=True
                )
                nc.vector.tensor_copy(S_sb[:, :], ps[:, :])
            nc.vector.tensor_reduce(
                out=cc_in[0:1, 0:1],
                in_=S_sb[0:1, 0:TILES],
                axis=mybir.AxisListType.X,
                op=add,
            )
            nc.vector.tensor_reduce(
                out=cc_in[0:1, 1:2],
                in_=S_sb[0:1, TILES : 2 * TILES],
                axis=mybir.AxisListType.X,
                op=add,
            )

            if use_collective:
                cc_in_d = dram.tile([1, 16], fp32, name="ccind")
                cc_out_d = dram.tile([1, 16], fp32, name="ccoutd")
                nc.gpsimd.dma_start(out=cc_in_d[:, :], in_=cc_in[:, :])
                nc.gpsimd.collective_compute(
                    "AllReduce",
                    mybir.AluOpType.add,
                    replica_groups=[list(range(CORES))],
                    ins=[cc_in_d.opt()],
                    outs=[cc_out_d.opt()],
                )
                AR = res.tile([1, 16], fp32, tag="AR", name="AR")
                nc.gpsimd.dma_start(out=AR[:, :], in_=cc_out_d[:, :])
                rec = res.tile([1, 1], fp32, tag="rec", name="rec")
                nc.vector.reciprocal(rec[:, :], AR[0:1, 1:2])
                lossv = res.tile([1, 1], fp32, tag="loss", name="loss")
                nc.vector.tensor_tensor(lossv[:, :], AR[0:1, 0:1], rec[:, :], mult)
                nc.vector.tensor_scalar_mul(lossv[:, :], lossv[:, :], -1.0)
                nc.sync.dma_start(out=out_d.ap()[:, :], in_=lossv[:, :])
            else:
                nc.sync.dma_start(out=out_d.ap()[:, :], in_=cc_in[0:1, 0:2])

    nc.compile()
    return nc


def _pack_o1t(out1):
    """out1.T quadrant-packed: per 2048-class chunk, classes [0,1024) go to
    partitions 0:64, classes [1024,2048) to partitions 64:128."""
    o1t = np.zeros((H1, 20 * 2048), dtype=np.float32)
    o1t[:, :C1] = out1.T
    o1t = o1t.reshape(H1, 20, 2, 1024).transpose(2, 0, 1, 3).reshape(128, 20 * 1024)
    return np.ascontiguousarray(o1t).astype(BF16)


def host_prep(x, target, head_w, proj0, out0, proj1, out1):
    """Class-sorted sharding + layout/dtype prep. Returns per-core in_maps."""
    x = np.asarray(x, dtype=np.float32)
    target = np.asarray(target).astype(np.int64)
    head_w = np.asarray(head_w, dtype=np.float32)
    proj0 = np.asarray(proj0, dtype=np.float32)
    out0 = np.asarray(out0, dtype=np.float32)
    proj1 = np.asarray(proj1, dtype=np.float32)
    out1 = np.asarray(out1, dtype=np.float32)

    valid = target != IGNORE
    tt = np.where(valid, target, 0)
    is_c0 = valid & (tt >= C0_LOW) & (tt < C0_HIGH)
    is_c1 = valid & (tt >= C0_HIGH)
    g = np.where(is_c1, 2001, np.where(is_c0, 2000, np.clip(tt, 0, C0_LOW - 1)))
    r0 = np.clip(tt - C0_LOW, 0, C0 - 1)
    r1 = np.clip(tt - C0_HIGH, 0, C1 - 1)

    idx_c1 = np.where(is_c1)[0]
    idx_c0 = np.where(is_c0)[0]
    idx_hd = np.where(~(is_c0 | is_c1))[0]
    assert len(idx_c1) <= CAP_C1, f"c1 overflow: {len(idx_c1)} > {CAP_C1}"
    assert len(idx_c0) <= CAP_C0, f"c0 overflow: {len(idx_c0)} > {CAP_C0}"
    assert len(idx_hd) <= CAP_HD, f"head overflow: {len(idx_hd)} > {CAP_HD}"

    def slots(idx, cap):
        a = np.full(cap, -1, dtype=np.int64)
        a[: len(idx)] = idx
        return a.reshape(CORES, cap // CORES)

    s1 = slots(idx_c1, CAP_C1)
    s0 = slots(idx_c0, CAP_C0)
    sh = slots(idx_hd, CAP_HD)
    pad = np.full((CORES, S_PAD), -1, dtype=np.int64)
    rows_per_core = np.concatenate([s1, pad, s0, sh], axis=1)  # [CORES, RPC]

    w_common = {
        "hwt": np.ascontiguousarray(head_w.T).astype(BF16).reshape(KT, 128, HEAD),
        "hwr": head_w.astype(BF16),
        "p0t": np.ascontiguousarray(proj0.T).astype(BF16).reshape(KT, 128, H0),
        "p1t": np.ascontiguousarray(proj1.T).astype(BF16).reshape(KT, 128, H1),
        "o0t": np.ascontiguousarray(out0.T).astype(BF16).reshape(2, 128, C0),
        "o0r": out0.astype(BF16),
        "o1t": _pack_o1t(out1),
        "o1r": out1.astype(BF16),
    }

    in_maps = []
    for c in range(CORES):
        rows = rows_per_core[c]
        real = rows >= 0
        rr = np.where(real, rows, 0)
        xs = x[rr] * real[:, None]
        m = dict(w_common)
        m["xt"] = np.ascontiguousarray(xs.T).astype(BF16).reshape(KT, 128, RPC)
        m["xr"] = xs.astype(BF16).reshape(TILES, 128, D)
        m["gidx"] = (g[rr] * real).astype(np.int32).reshape(TILES, 128, 1)
        m["r0idx"] = (r0[rr] * real).astype(np.int32).reshape(TILES, 128, 1)
        m["r1idx"] = (r1[rr] * real).astype(np.int32).reshape(TILES, 128, 1)

        def pt(v):
            return np.ascontiguousarray(v.astype(np.float32).reshape(TILES, 128).T)

        m["validm"] = pt(valid[rr] & real)
        m["c0m"] = pt(is_c0[rr] & real)
        m["c1m"] = pt(is_c1[rr] & real)
        in_maps.append(m)
    return in_maps


def kernel(x, target, head_w, proj0, out0, proj1, out1):
    _install_profile_hook()
    from concourse.bass_utils import run_bass_kernel_spmd

    use_collective = False
    key = ("nc", use_collective)
    if key not in _CACHE:
        _CACHE[key] = build_nc(use_collective)
    nc = _CACHE[key]

    in_maps = host_prep(x, target, head_w, proj0, out0, proj1, out1)
    res = run_bass_kernel_spmd(nc, in_maps, core_ids=list(range(CORES)), trace=False)
    if use_collective:
        loss = np.float32(res.results[0]["out"][0, 0])
    else:
        acc = np.zeros(2, dtype=np.float64)
        for r in res.results:
            acc += r["out"][0].astype(np.float64)
        loss = np.float32(-acc[0] / acc[1])
    return np.asarray(loss, dtype=np.float32)


if __name__ == "__main__":
    rng = np.random.default_rng(0)
    x = rng.standard_normal((N_ROWS, D), dtype=np.float32)
    target = rng.integers(0, NCLS, size=(N_ROWS,)).astype(np.int64)
    target[rng.random(N_ROWS) < 0.1] = IGNORE
    s = 1.0 / np.sqrt(D)
    head_w = rng.standard_normal((HEAD, D), dtype=np.float32) * s
    proj0 = rng.standard_normal((H0, D), dtype=np.float32) * s
    out0 = rng.standard_normal((C0, H0), dtype=np.float32) / np.sqrt(H0)
    proj1 = rng.standard_normal((H1, D), dtype=np.float32) * s
    out1 = rng.standard_normal((C1, H1), dtype=np.float32) / np.sqrt(H1)
    print(kernel(x=x, target=target, head_w=head_w, proj0=proj0, out0=out0,
                 proj1=proj1, out1=out1))


# revision 12
# speedup vs baseline: 1.1795x; 1.1795x over previous
"""AdaptiveLogSoftmaxWithLoss (with ignore_index) on 8 TRN2 NeuronCores.

Strategy (data-parallel over the token axis, with host-side class-sorted
row partitioning):
  - Rows are bucketed by target class band (cluster1 / cluster0 /
    head-or-ignored), padded to fixed per-core section sizes, and sharded
    so every core gets the same per-section row counts:
        [c1: 448][zero-pad: 64][c0: 128][head-only: 128]  = 768 rows/core.
  - The expensive per-band partition sums then run only on the rows that
    need them: head logsumexp over all 768 rows, cluster0 (8000 classes)
    over 128 rows, cluster1 (40257 classes) over 448 rows.
  - Weights are replicated (bf16, pre-transposed on host).
  - Per-row target logits come from an indirect-DMA gather of the target's
    weight row + a multiply-reduce dot, so full logits are never gathered.
  - Per-core masked loss numerator + valid count are returned and combined
    on the host (the 8-way sum + divide).

kernel(**inputs) takes the FULL unsharded inputs and returns the scalar loss.
"""

import sys
import types

import ml_dtypes
import numpy as np

# ---------------------------------------------------------------- constants
CORES = 8
N_ROWS = 4096
D = 1024
KT = D // 128                  # k tiles = 8
HEAD = 2002                    # head classes (2000 + 2 cluster slots)
C0_LOW, C0_HIGH, NCLS = 2000, 10000, 50257
C0 = C0_HIGH - C0_LOW          # 8000
C1 = NCLS - C0_HIGH            # 40257
H0, H1 = 256, 64
IGNORE = -1

# class-sorted layout (per core)
S_C1, S_PAD, S_C0, S_HD = 448, 64, 128, 128
RPC = S_C1 + S_PAD + S_C0 + S_HD          # 768 rows per core
TILES = RPC // 128                        # 6 row tiles per core
CAP_C1, CAP_C0, CAP_HD = S_C1 * CORES, S_C0 * CORES, S_HD * CORES

HEAD_TILES = [(t, 128) for t in range(TILES)]
C0_TILES = [(4, 128)]
C1_TILES = [(0, 128), (1, 128), (2, 128), (3, 64)]

BF16 = ml_dtypes.bfloat16

_CACHE = {}


def _install_profile_hook():
    """Register the axon NTFF profile hook (missing from the image's antenv)
    so run_bass_kernel_spmd(trace=True) can return exec_time_ns."""
    if "antenv.axon_hooks" in sys.modules:
        return
    try:
        mod = types.ModuleType("antenv.axon_hooks")
        state = {"hook": None}
        mod.set_axon_ntff_profile_hook = lambda h: state.update(hook=h)
        mod.get_axon_ntff_profile_hook = lambda: state["hook"]
        sys.modules["antenv.axon_hooks"] = mod
        import antenv

        antenv.axon_hooks = mod
        from trn_agent_boot.trn_boot import _ntff_profile_via_ctypes

        mod.set_axon_ntff_profile_hook(
            _ntff_profile_via_ctypes("/opt/axon/libaxon_pjrt.so")
        )
        from concourse import bass_utils

        bass_utils.upload_artifacts = lambda tmpdir: tmpdir
    except Exception:
        pass


def _enable_ldw_opt():
    """Flip walrus --enable-ldw-opt to true: consecutive matmuls that reuse
    the same stationary operand then skip the redundant LDWEIGHTS."""
    from concourse import bass_utils

    if getattr(bass_utils, "_ldw_patched", False):
        return
    orig = bass_utils.run_command

    def patched(cmd, **kw):
        cmd = [
            "--enable-ldw-opt=true" if c == "--enable-ldw-opt=false" else c
            for c in cmd
        ]
        return orig(cmd, **kw)

    bass_utils.run_command = patched
    bass_utils._ldw_patched = True


def _chunks(total, size):
    return [(a, min(a + size, total)) for a in range(0, total, size)]


def build_nc(use_collective=False, act_span=2048):
    from concourse import bacc, bass, mybir, tile

    fp32 = mybir.dt.float32
    bf16 = mybir.dt.bfloat16
    i32 = mybir.dt.int32

    nc = bacc.Bacc("TRN2", target_bir_lowering=False, debug=False, num_devices=CORES)
    # Keep matmuls un-split (no standalone InstLdweights) so walrus'
    # ldw-opt can elide redundant weight loads for consecutive matmuls
    # that share the same stationary operand.
    nc.move_matmul_waits_to_ldweights = lambda: None

    # ------------------------------------------------------------ parameters
    def param(name, shape, dt):
        return nc.declare_dram_parameter(name, list(shape), dt, isOutput=False)

    xt_d = param("xt", [KT, 128, RPC], bf16)           # x shard, transposed
    xr_d = param("xr", [TILES, 128, D], bf16)          # x shard, row-major
    gidx_d = param("gidx", [TILES, 128, 1], i32)       # head gather row idx
    r0idx_d = param("r0idx", [TILES, 128, 1], i32)
    r1idx_d = param("r1idx", [TILES, 128, 1], i32)
    valid_d = param("validm", [128, TILES], fp32)      # [p, t] masks
    c0m_d = param("c0m", [128, TILES], fp32)
    c1m_d = param("c1m", [128, TILES], fp32)
    hwt_d = param("hwt", [KT, 128, HEAD], bf16)        # head_w.T k-tiles
    hwr_d = param("hwr", [HEAD, D], bf16)              # head_w row-major
    p0t_d = param("p0t", [KT, 128, H0], bf16)
    p1t_d = param("p1t", [KT, 128, H1], bf16)
    o0t_d = param("o0t", [2, 128, C0], bf16)           # out0.T k-tiles
    o0r_d = param("o0r", [C0, H0], bf16)
    o1t_d = param("o1t", [128, 20 * 1024], bf16)       # out1.T quadrant-packed
    o1r_d = param("o1r", [C1, H1], bf16)

    out_shape = [1, 1] if use_collective else [1, 2]
    out_d = nc.declare_dram_parameter("out", out_shape, fp32, isOutput=True)

    head_slices = _chunks(HEAD, 512)
    c0_chunks = _chunks(C0, 2048)       # 4 chunks
    c1_chunks = _chunks(C1, 2048)       # 20 chunks

    def n_parts(total):
        return sum(len(_chunks(b - a, act_span)) for a, b in _chunks(total, 2048))

    nh, nc0p, nc1p = len(_chunks(HEAD, 512)), n_parts(C0), n_parts(C1)

    with tile.TileContext(nc) as tc:
        with (
            tc.tile_pool(name="res", bufs=1) as res,       # resident SBUF
            tc.tile_pool(name="wstream", bufs=2) as wst,   # streamed weights
            tc.tile_pool(name="scr", bufs=1) as scr,       # ACT exp scratch
            tc.tile_pool(name="dram", bufs=1, space="DRAM") as dram,
        ):
            # -------------------------------------------- resident loads
            # Order matters: it is both the DMA queue order and (roughly) the
            # per-engine program order. xt + head slices first so the head
            # matmul pipeline starts as soon as (k=0, s=0) land.
            xt_sb = []
            for k in range(KT):
                t_ = res.tile([128, RPC], bf16, tag=f"xt{k}", name=f"xt{k}")
                nc.sync.dma_start(out=t_[:, :], in_=xt_d.ap()[k])
                xt_sb.append(t_)
            p0t_sb, p1t_sb = [], []
            for k in range(KT):
                a = res.tile([128, H0], bf16, tag=f"p0t{k}", name=f"p0t{k}")
                nc.sync.dma_start(out=a[:, :], in_=p0t_d.ap()[k])
                p0t_sb.append(a)
                b = res.tile([128, H1], bf16, tag=f"p1t{k}", name=f"p1t{k}")
                nc.sync.dma_start(out=b[:, :], in_=p1t_d.ap()[k])
                p1t_sb.append(b)
            # prefetch the first cluster-1 weight chunks so the c1 pipeline
            # starts before the bulk resident loads finish
            N_W1_PREF = 3
            w1_tiles = {}
            for ci in range(N_W1_PREF):
                w = wst.tile([128, 1024], bf16, tag="w1", name="w1", bufs=N_W1_PREF + 2)
                nc.sync.dma_start(
                    out=w[:, :], in_=o1t_d.ap()[:, 1024 * ci : 1024 * (ci + 1)]
                )
                w1_tiles[ci] = w
            hwt_sb = {}
            for k in range(KT):
                for si, (s0, s1) in enumerate(head_slices):
                    t_ = res.tile(
                        [128, s1 - s0], bf16, tag=f"hwt{k}_{si}", name=f"hwt{k}_{si}"
                    )
                    nc.sync.dma_start(out=t_[:, :], in_=hwt_d.ap()[k, :, s0:s1])
                    hwt_sb[(k, si)] = t_
            xr_sb = []
            for t in range(TILES):
                t_ = res.tile([128, D], bf16, tag=f"xr{t}", name=f"xr{t}")
                nc.sync.dma_start(out=t_[:, :], in_=xr_d.ap()[t])
                xr_sb.append(t_)
            idx_sb = {}
            for name, d in (("g", gidx_d), ("r0", r0idx_d), ("r1", r1idx_d)):
                for t in range(TILES):
                    t_ = res.tile(
                        [128, 1], i32, tag=f"idx{name}{t}", name=f"idx{name}{t}"
                    )
                    nc.sync.dma_start(out=t_[:, :], in_=d.ap()[t])
                    idx_sb[(name, t)] = t_
            valid_sb = res.tile([128, TILES], fp32, tag="valid", name="valid")
            c0m_sb = res.tile([128, TILES], fp32, tag="c0m", name="c0m")
            c1m_sb = res.tile([128, TILES], fp32, tag="c1m", name="c1m")
            nc.sync.dma_start(out=valid_sb[:, :], in_=valid_d.ap()[:, :])
            nc.sync.dma_start(out=c0m_sb[:, :], in_=c0m_d.ap()[:, :])
            nc.sync.dma_start(out=c1m_sb[:, :], in_=c1m_d.ap()[:, :])

            # -------------------------------------------- indirect gathers
            hwg_sb, o0g_sb, o1g_sb = [], [], []
            for t in range(TILES):
                g = res.tile([128, D], bf16, tag=f"hwg{t}", name=f"hwg{t}")
                nc.gpsimd.indirect_dma_start(
                    out=g[:, :], out_offset=None, in_=hwr_d.ap()[:, :],
                    in_offset=bass.IndirectOffsetOnAxis(
                        ap=idx_sb[("g", t)][:, :1], axis=0
                    ),
                )
                hwg_sb.append(g)
                g0 = res.tile([128, H0], bf16, tag=f"o0g{t}", name=f"o0g{t}")
                nc.gpsimd.indirect_dma_start(
                    out=g0[:, :], out_offset=None, in_=o0r_d.ap()[:, :],
                    in_offset=bass.IndirectOffsetOnAxis(
                        ap=idx_sb[("r0", t)][:, :1], axis=0
                    ),
                )
                o0g_sb.append(g0)
                g1 = res.tile([128, H1], bf16, tag=f"o1g{t}", name=f"o1g{t}")
                nc.gpsimd.indirect_dma_start(
                    out=g1[:, :], out_offset=None, in_=o1r_d.ap()[:, :],
                    in_offset=bass.IndirectOffsetOnAxis(
                        ap=idx_sb[("r1", t)][:, :1], axis=0
                    ),
                )
                o1g_sb.append(g1)

            # -------------------------------------------- SBUF result tiles
            p0T_sb = res.tile([128, S_C0], bf16, tag="p0T_a", name="p0T_a")
            p0T_sb2 = res.tile([128, S_C0], bf16, tag="p0T_b", name="p0T_b")
            p1T_sb = res.tile([128, S_C1 + S_PAD], bf16, tag="p1T", name="p1T")
            p0r_sb = {}
            p1r_sb = {}
            for t, m in C0_TILES:
                p0r_sb[t] = res.tile([128, H0], bf16, tag=f"p0r{t}", name=f"p0r{t}")
            for t, m in C1_TILES:
                p1r_sb[t] = res.tile([128, H1], bf16, tag=f"p1r{t}", name=f"p1r{t}")
            hd_sb = res.tile([128, TILES], fp32, tag="hd", name="hd")
            cd0_sb = res.tile([128, TILES], fp32, tag="cd0", name="cd0")
            cd1_sb = res.tile([128, TILES], fp32, tag="cd1", name="cd1")
            F_sb = res.tile([128, 3 * TILES], fp32, tag="F", name="F")
            zh_parts = res.tile([128, TILES * nh], fp32, tag="zhp", name="zhp")
            zc0_parts = res.tile([128, TILES * nc0p], fp32, tag="zc0p", name="zc0p")
            zc1_parts = res.tile([128, TILES * nc1p], fp32, tag="zc1p", name="zc1p")
            exp_scr = scr.tile([128, 2048], bf16, tag="exp", name="exp")

            # sections only write their own tiles/partitions; everything else
            # must be a harmless finite value (exp-sum 1.0 -> Ln finite).
            nc.gpsimd.memset(zc0_parts[:, :], 1.0)
            nc.gpsimd.memset(zc1_parts[:, :], 1.0)
            nc.gpsimd.memset(zh_parts[:, :], 1.0)
            nc.gpsimd.memset(cd0_sb[:, :], 0.0)
            nc.gpsimd.memset(cd1_sb[:, :], 0.0)

            def act_exp(ps, m, cw, parts, t, npart, base):
                """exp + accumulate psum[:m, :cw] in act_span pieces."""
                for j, (a0, a1) in enumerate(_chunks(cw, act_span)):
                    col = t * npart + base + j
                    nc.scalar.activation(
                        exp_scr[:m, a0:a1],
                        ps[:m, a0:a1],
                        mybir.ActivationFunctionType.Exp,
                        accum_out=parts[:m, col : col + 1],
                    )

            with tc.tile_pool(name="big", bufs=2, space="PSUM") as bigp:

                def big_psum():
                    return bigp.tile([128, 2048], fp32, tag="big", name="big")

                # ---------------- projections first (warms PE, unblocks c1)
                # p1T: the c1 section rows (tiles 0..3 incl pad)
                ps = big_psum()
                for k in range(KT):
                    nc.tensor.matmul(
                        ps[:64, : S_C1 + S_PAD],
                        lhsT=p1t_sb[k][:, :],
                        rhs=xt_sb[k][:, : S_C1 + S_PAD],
                        start=(k == 0),
                        stop=(k == KT - 1),
                    )
                nc.vector.tensor_copy(p1T_sb[0:64, :], ps[:64, : S_C1 + S_PAD])
                nc.vector.tensor_copy(p1T_sb[64:128, :], ps[:64, : S_C1 + S_PAD])
                # p0T: only the c0 section rows (tile 4)
                c0_t0 = C0_TILES[0][0]
                for mi, dst in ((0, p0T_sb), (1, p0T_sb2)):
                    ps = big_psum()
                    for k in range(KT):
                        nc.tensor.matmul(
                            ps[:, : S_C0],
                            lhsT=p0t_sb[k][:, 128 * mi : 128 * (mi + 1)],
                            rhs=xt_sb[k][:, 128 * c0_t0 : 128 * c0_t0 + S_C0],
                            start=(k == 0),
                            stop=(k == KT - 1),
                        )
                    nc.vector.tensor_copy(dst[:, :], ps[:, : S_C0])
                # row-major projections for the dot products
                for t, m in C0_TILES:
                    ps = big_psum()
                    for k in range(KT):
                        nc.tensor.matmul(
                            ps[:m, :H0],
                            lhsT=xt_sb[k][:, 128 * t : 128 * t + m],
                            rhs=p0t_sb[k][:, :],
                            start=(k == 0),
                            stop=(k == KT - 1),
                        )
                    nc.vector.tensor_copy(p0r_sb[t][:m, :], ps[:m, :H0])
                for t, m in C1_TILES:
                    ps = big_psum()
                    for k in range(KT):
                        nc.tensor.matmul(
                            ps[:m, :H1],
                            lhsT=xt_sb[k][:, 128 * t : 128 * t + m],
                            rhs=p1t_sb[k][:, :],
                            start=(k == 0),
                            stop=(k == KT - 1),
                        )
                    nc.vector.tensor_copy(p1r_sb[t][:m, :], ps[:m, :H1])

                # ---------------- merged schedule: c1 (chunk, tile) units
                # paced with fine-grained head/c0 fillers. c1 is ACT-heavy /
                # PE-light; head is the reverse. Head fillers are single
                # 512-col slices so no unit holds a PSUM slot for long.
                def emit_head_slice(t, m, si):
                    s0, s1 = head_slices[si]
                    ps = big_psum()
                    for k in range(KT):
                        nc.tensor.matmul(
                            ps[:m, : s1 - s0],
                            lhsT=xt_sb[k][:, 128 * t : 128 * t + m],
                            rhs=hwt_sb[(k, si)][:, :],
                            start=(k == 0),
                            stop=(k == KT - 1),
                        )
                    nc.scalar.activation(
                        exp_scr[:m, : s1 - s0],
                        ps[:m, : s1 - s0],
                        mybir.ActivationFunctionType.Exp,
                        accum_out=zh_parts[:m, t * nh + si : t * nh + si + 1],
                    )

                def emit_c0_chunk(ci):
                    c0, c1_ = c0_chunks[ci]
                    w = [
                        wst.tile([128, 2048], bf16, tag="w0a", name="w0a"),
                        wst.tile([128, 2048], bf16, tag="w0b", name="w0b"),
                    ]
                    cw = c1_ - c0
                    for k in range(2):
                        nc.sync.dma_start(
                            out=w[k][:, :cw], in_=o0t_d.ap()[k, :, c0:c1_]
                        )
                    pbase = sum(
                        len(_chunks(b - a, act_span)) for a, b in c0_chunks[:ci]
                    )
                    for t, m in C0_TILES:
                        ps = big_psum()
                        p0T = (p0T_sb, p0T_sb2)
                        for k in range(2):
                            for s0, s1 in _chunks(cw, 512):
                                nc.tensor.matmul(
                                    ps[:m, s0:s1],
                                    lhsT=p0T[k][:, :m],
                                    rhs=w[k][:, s0:s1],
                                    start=(k == 0),
                                    stop=(k == 1),
                                )
                        act_exp(ps, m, cw, zc0_parts, t, nc0p, pbase)

                fillers = [
                    ("head", t, si)
                    for t, m in HEAD_TILES
                    for si in range(len(head_slices))
                ] + [("c0", ci, 0) for ci in range(len(c0_chunks))]
                n_units = len(c1_chunks) * len(C1_TILES)
                fper = len(fillers) / n_units
                fcredit = 0.0
                fi = 0
                for ci in range(len(c1_chunks)):
                    c0, c1_ = c1_chunks[ci]
                    cw = c1_ - c0          # real classes this chunk (<= 2048)
                    if ci in w1_tiles:
                        w = w1_tiles.pop(ci)
                    else:
                        w = wst.tile(
                            [128, 1024], bf16, tag="w1", name="w1", bufs=N_W1_PREF + 2
                        )
                        nc.sync.dma_start(
                            out=w[:, :], in_=o1t_d.ap()[:, 1024 * ci : 1024 * (ci + 1)]
                        )
                    pbase = sum(
                        len(_chunks(b - a, act_span)) for a, b in c1_chunks[:ci]
                    )
                    for t, m in C1_TILES:
                        ps = big_psum()
                        # quadrant-packed: top half of w = classes [0,1024),
                        # bottom half = classes [1024,2048) of this chunk.
                        # Alternate quadrants so LDWEIGHTS overlaps matmuls.
                        mms_a = [(0, s0, s1) for s0, s1 in _chunks(min(cw, 1024), 512)]
                        mms_b = (
                            [(1, s0, s1) for s0, s1 in _chunks(cw - 1024, 512)]
                            if cw > 1024
                            else []
                        )
                        order = []
                        for i in range(max(len(mms_a), len(mms_b))):
                            if i < len(mms_a):
                                order.append(mms_a[i])
                            if i < len(mms_b):
                                order.append(mms_b[i])
                        for h, s0, s1 in order:
                            nc.tensor.matmul(
                                ps[:m, 1024 * h + s0 : 1024 * h + s1],
                                lhsT=p1T_sb[64 * h : 64 * h + 64, 128 * t : 128 * t + m],
                                rhs=w[64 * h : 64 * h + 64, s0:s1],
                                start=True,
                                stop=True,
                            )
                        act_exp(ps, m, cw, zc1_parts, t, nc1p, pbase)
                        fcredit += fper
                        while fcredit >= 1.0 and fi < len(fillers):
                            kind, a, b = fillers[fi]
                            fi += 1
                            fcredit -= 1.0
                            if kind == "head":
                                emit_head_slice(a, 128, b)
                            else:
                                emit_c0_chunk(a)
                while fi < len(fillers):
                    kind, a, b = fillers[fi]
                    fi += 1
                    if kind == "head":
                        emit_head_slice(a, 128, b)
                    else:
                        emit_c0_chunk(a)

                # ---------------- dots (DVE, off critical path)
                dots_scr = scr.tile([128, D], bf16, tag="dots", name="dots")
                for t in range(TILES):
                    nc.vector.tensor_tensor(
                        dots_scr[:, :D], xr_sb[t][:, :], hwg_sb[t][:, :],
                        mybir.AluOpType.mult,
                    )
                    nc.vector.tensor_reduce(
                        out=hd_sb[:, t : t + 1], in_=dots_scr[:, :D],
                        axis=mybir.AxisListType.X, op=mybir.AluOpType.add,
                    )
                for t, m in C0_TILES:
                    nc.vector.tensor_tensor(
                        dots_scr[:m, :H0], p0r_sb[t][:m, :], o0g_sb[t][:m, :],
                        mybir.AluOpType.mult,
                    )
                    nc.vector.tensor_reduce(
                        out=cd0_sb[:m, t : t + 1], in_=dots_scr[:m, :H0],
                        axis=mybir.AxisListType.X, op=mybir.AluOpType.add,
                    )
                for t, m in C1_TILES:
                    nc.vector.tensor_tensor(
                        dots_scr[:m, :H1], p1r_sb[t][:m, :], o1g_sb[t][:m, :],
                        mybir.AluOpType.mult,
                    )
                    nc.vector.tensor_reduce(
                        out=cd1_sb[:m, t : t + 1], in_=dots_scr[:m, :H1],
                        axis=mybir.AxisListType.X, op=mybir.AluOpType.add,
                    )

            # -------------------------------------------- per-row loss
            for t in range(TILES):
                for sec_i, (parts, npart) in enumerate(
                    ((zh_parts, nh), (zc0_parts, nc0p), (zc1_parts, nc1p))
                ):
                    nc.vector.tensor_reduce(
                        out=F_sb[:, sec_i * TILES + t : sec_i * TILES + t + 1],
                        in_=parts[:, t * npart : (t + 1) * npart],
                        axis=mybir.AxisListType.X,
                        op=mybir.AluOpType.add,
                    )
            L_sb = res.tile([128, 3 * TILES], fp32, tag="L", name="L")
            nc.scalar.activation(
                L_sb[:, :], F_sb[:, :], mybir.ActivationFunctionType.Ln
            )
            T1 = res.tile([128, TILES], fp32, tag="T1", name="T1")
            T2 = res.tile([128, TILES], fp32, tag="T2", name="T2")
            R_sb = res.tile([128, 2 * TILES], bf16, tag="R", name="R")
            sub, mult, add = (
                mybir.AluOpType.subtract,
                mybir.AluOpType.mult,
                mybir.AluOpType.add,
            )
            nc.vector.tensor_tensor(T1[:, :], hd_sb[:, :], L_sb[:, 0:TILES], sub)
            nc.vector.tensor_tensor(
                T2[:, :], cd0_sb[:, :], L_sb[:, TILES : 2 * TILES], sub
            )
            nc.vector.tensor_tensor(T2[:, :], T2[:, :], c0m_sb[:, :], mult)
            nc.vector.tensor_tensor(T1[:, :], T1[:, :], T2[:, :], add)
            nc.vector.tensor_tensor(
                T2[:, :], cd1_sb[:, :], L_sb[:, 2 * TILES : 3 * TILES], sub
            )
            nc.vector.tensor_tensor(T2[:, :], T2[:, :], c1m_sb[:, :], mult)
            nc.vector.tensor_tensor(T1[:, :], T1[:, :], T2[:, :], add)
            nc.vector.tensor_tensor(R_sb[:, 0:TILES], T1[:, :], valid_sb[:, :], mult)
            nc.vector.tensor_copy(R_sb[:, TILES : 2 * TILES], valid_sb[:, :])

            # -------------------------------------------- reduce across rows
            ones_sb = res.tile([128, 1], bf16, tag="ones", name="ones")
            nc.gpsimd.memset(ones_sb[:, :], 1.0)
            S_sb = res.tile([1, 2 * TILES], fp32, tag="S", name="S")
            cc_in = res.tile([1, 16], fp32, tag="ccin", name="ccin")
            nc.gpsimd.memset(cc_in[:, :], 0.0)
            with tc.tile_pool(name="sm", bufs=1, space="PSUM") as smp:
                ps = smp.tile([1, 2 * TILES], fp32, tag="sm", name="sm")
                nc.tensor.matmul(
                    ps[:, :], lhsT=ones_sb[:, :], rhs=R_sb[:, :], start=True, stop=True
                )
                nc.vector.tensor_copy(S_sb[:, :], ps[:, :])
            nc.vector.tensor_reduce(
                out=cc_in[0:1, 0:1],
                in_=S_sb[0:1, 0:TILES],
                axis=mybir.AxisListType.X,
                op=add,
            )
            nc.vector.tensor_reduce(
                out=cc_in[0:1, 1:2],
                in_=S_sb[0:1, TILES : 2 * TILES],
                axis=mybir.AxisListType.X,
                op=add,
            )

            if use_collective:
                cc_in_d = dram.tile([1, 16], fp32, name="ccind")
                cc_out_d = dram.tile([1, 16], fp32, name="ccoutd")
                nc.gpsimd.dma_start(out=cc_in_d[:, :], in_=cc_in[:, :])
                nc.gpsimd.collective_compute(
                    "AllReduce",
                    mybir.AluOpType.add,
                    replica_groups=[list(range(CORES))],
                    ins=[cc_in_d.opt()],
                    outs=[cc_out_d.opt()],
                )
                AR = res.tile([1, 16], fp32, tag="AR", name="AR")
                nc.gpsimd.dma_start(out=AR[:, :], in_=cc_out_d[:, :])
                rec = res.tile([1, 1], fp32, tag="rec", name="rec")
                nc.vector.reciprocal(rec[:, :], AR[0:1, 1:2])
                lossv = res.tile([1, 1], fp32, tag="loss", name="loss")
                nc.vector.tensor_tensor(lossv[:, :], AR[0:1, 0:1], rec[:, :], mult)
                nc.vector.tensor_scalar_mul(lossv[:, :], lossv[:, :], -1.0)
                nc.sync.dma_start(out=out_d.ap()[:, :], in_=lossv[:, :])
            else:
                nc.sync.dma_start(out=out_d.ap()[:, :], in_=cc_in[0:1, 0:2])

    nc.compile()
    return nc


def _pack_o1t(out1):
    """out1.T quadrant-packed: per 2048-class chunk, classes [0,1024) go to
    partitions 0:64, classes [1024,2048) to partitions 64:128."""
    o1t = np.zeros((H1, 20 * 2048), dtype=np.float32)
    o1t[:, :C1] = out1.T
    o1t = o1t.reshape(H1, 20, 2, 1024).transpose(2, 0, 1, 3).reshape(128, 20 * 1024)
    return np.ascontiguousarray(o1t).astype(BF16)


def host_prep(x, target, head_w, proj0, out0, proj1, out1):
    """Class-sorted sharding + layout/dtype prep. Returns per-core in_maps."""
    x = np.asarray(x, dtype=np.float32)
    target = np.asarray(target).astype(np.int64)
    head_w = np.asarray(head_w, dtype=np.float32)
    proj0 = np.asarray(proj0, dtype=np.float32)
    out0 = np.asarray(out0, dtype=np.float32)
    proj1 = np.asarray(proj1, dtype=np.float32)
    out1 = np.asarray(out1, dtype=np.float32)

    valid = target != IGNORE
    tt = np.where(valid, target, 0)
    is_c0 = valid & (tt >= C0_LOW) & (tt < C0_HIGH)
    is_c1 = valid & (tt >= C0_HIGH)
    g = np.where(is_c1, 2001, np.where(is_c0, 2000, np.clip(tt, 0, C0_LOW - 1)))
    r0 = np.clip(tt - C0_LOW, 0, C0 - 1)
    r1 = np.clip(tt - C0_HIGH, 0, C1 - 1)

    idx_c1 = np.where(is_c1)[0]
    idx_c0 = np.where(is_c0)[0]
    idx_hd = np.where(~(is_c0 | is_c1))[0]
    assert len(idx_c1) <= CAP_C1, f"c1 overflow: {len(idx_c1)} > {CAP_C1}"
    assert len(idx_c0) <= CAP_C0, f"c0 overflow: {len(idx_c0)} > {CAP_C0}"
    assert len(idx_hd) <= CAP_HD, f"head overflow: {len(idx_hd)} > {CAP_HD}"

    def slots(idx, cap):
        a = np.full(cap, -1, dtype=np.int64)
        a[: len(idx)] = idx
        return a.reshape(CORES, cap // CORES)

    s1 = slots(idx_c1, CAP_C1)
    s0 = slots(idx_c0, CAP_C0)
    sh = slots(idx_hd, CAP_HD)
    pad = np.full((CORES, S_PAD), -1, dtype=np.int64)
    rows_per_core = np.concatenate([s1, pad, s0, sh], axis=1)  # [CORES, RPC]

    w_common = {
        "hwt": np.ascontiguousarray(head_w.T).astype(BF16).reshape(KT, 128, HEAD),
        "hwr": head_w.astype(BF16),
        "p0t": np.ascontiguousarray(proj0.T).astype(BF16).reshape(KT, 128, H0),
        "p1t": np.ascontiguousarray(proj1.T).astype(BF16).reshape(KT, 128, H1),
        "o0t": np.ascontiguousarray(out0.T).astype(BF16).reshape(2, 128, C0),
        "o0r": out0.astype(BF16),
        "o1t": _pack_o1t(out1),
        "o1r": out1.astype(BF16),
    }

    in_maps = []
    for c in range(CORES):
        rows = rows_per_core[c]
        real = rows >= 0
        rr = np.where(real, rows, 0)
        xs = x[rr] * real[:, None]
        m = dict(w_common)
        m["xt"] = np.ascontiguousarray(xs.T).astype(BF16).reshape(KT, 128, RPC)
        m["xr"] = xs.astype(BF16).reshape(TILES, 128, D)
        m["gidx"] = (g[rr] * real).astype(np.int32).reshape(TILES, 128, 1)
        m["r0idx"] = (r0[rr] * real).astype(np.int32).reshape(TILES, 128, 1)
        m["r1idx"] = (r1[rr] * real).astype(np.int32).reshape(TILES, 128, 1)

        def pt(v):
            return np.ascontiguousarray(v.astype(np.float32).reshape(TILES, 128).T)

        m["validm"] = pt(valid[rr] & real)
        m["c0m"] = pt(is_c0[rr] & real)
        m["c1m"] = pt(is_c1[rr] & real)
        in_maps.append(m)
    return in_maps


def kernel(x, target, head_w, proj0, out0, proj1, out1):
    _install_profile_hook()
    from concourse.bass_utils import run_bass_kernel_spmd

    use_collective = False
    key = ("nc", use_collective)
    if key not in _CACHE:
        _CACHE[key] = build_nc(use_collective)
    nc = _CACHE[key]

    in_maps = host_prep(x, target, head_w, proj0, out0, proj1, out1)
    res = run_bass_kernel_spmd(nc, in_maps, core_ids=list(range(CORES)), trace=False)
    if use_collective:
        loss = np.float32(res.results[0]["out"][0, 0])
    else:
        acc = np.zeros(2, dtype=np.float64)
        for r in res.results:
            acc += r["out"][0].astype(np.float64)
        loss = np.float32(-acc[0] / acc[1])
    return np.asarray(loss, dtype=np.float32)


if __name__ == "__main__":
    rng = np.random.default_rng(0)
    x = rng.standard_normal((N_ROWS, D), dtype=np.float32)
    target = rng.integers(0, NCLS, size=(N_ROWS,)).astype(np.int64)
    target[rng.random(N_ROWS) < 0.1] = IGNORE
    s = 1.0 / np.sqrt(D)
    head_w = rng.standard_normal((HEAD, D), dtype=np.float32) * s
    proj0 = rng.standard_normal((H0, D), dtype=np.float32) * s
    out0 = rng.standard_normal((C0, H0), dtype=np.float32) / np.sqrt(H0)
    proj1 = rng.standard_normal((H1, D), dtype=np.float32) * s
    out1 = rng.standard_normal((C1, H1), dtype=np.float32) / np.sqrt(H1)
    print(kernel(x=x, target=target, head_w=head_w, proj0=proj0, out0=out0,
                 proj1=proj1, out1=out1))


# revision 13
# speedup vs baseline: 1.1799x; 1.0003x over previous
"""AdaptiveLogSoftmaxWithLoss (with ignore_index) on 8 TRN2 NeuronCores.

Strategy (data-parallel over the token axis, with host-side class-sorted
row partitioning):
  - Rows are bucketed by target class band (cluster1 / cluster0 /
    head-or-ignored), padded to fixed per-core section sizes, and sharded
    so every core gets the same per-section row counts:
        [c1: 448][zero-pad: 64][c0: 128][head-only: 128]  = 768 rows/core.
  - The expensive per-band partition sums then run only on the rows that
    need them: head logsumexp over all 768 rows, cluster0 (8000 classes)
    over 128 rows, cluster1 (40257 classes) over 448 rows.
  - Weights are replicated (bf16, pre-transposed on host).
  - Per-row target logits come from an indirect-DMA gather of the target's
    weight row + a multiply-reduce dot, so full logits are never gathered.
  - Per-core masked loss numerator + valid count are returned and combined
    on the host (the 8-way sum + divide).

kernel(**inputs) takes the FULL unsharded inputs and returns the scalar loss.
"""

import sys
import types

import ml_dtypes
import numpy as np

# ---------------------------------------------------------------- constants
CORES = 8
N_ROWS = 4096
D = 1024
KT = D // 128                  # k tiles = 8
HEAD = 2002                    # head classes (2000 + 2 cluster slots)
C0_LOW, C0_HIGH, NCLS = 2000, 10000, 50257
C0 = C0_HIGH - C0_LOW          # 8000
C1 = NCLS - C0_HIGH            # 40257
H0, H1 = 256, 64
IGNORE = -1

# class-sorted layout (per core)
S_C1, S_PAD, S_C0, S_HD = 448, 64, 128, 128
RPC = S_C1 + S_PAD + S_C0 + S_HD          # 768 rows per core
TILES = RPC // 128                        # 6 row tiles per core
CAP_C1, CAP_C0, CAP_HD = S_C1 * CORES, S_C0 * CORES, S_HD * CORES

HEAD_TILES = [(t, 128) for t in range(TILES)]
C0_TILES = [(4, 128)]
C1_TILES = [(0, 128), (1, 128), (2, 128), (3, 64)]

BF16 = ml_dtypes.bfloat16

_CACHE = {}


def _install_profile_hook():
    """Register the axon NTFF profile hook (missing from the image's antenv)
    so run_bass_kernel_spmd(trace=True) can return exec_time_ns."""
    if "antenv.axon_hooks" in sys.modules:
        return
    try:
        mod = types.ModuleType("antenv.axon_hooks")
        state = {"hook": None}
        mod.set_axon_ntff_profile_hook = lambda h: state.update(hook=h)
        mod.get_axon_ntff_profile_hook = lambda: state["hook"]
        sys.modules["antenv.axon_hooks"] = mod
        import antenv

        antenv.axon_hooks = mod
        from trn_agent_boot.trn_boot import _ntff_profile_via_ctypes

        mod.set_axon_ntff_profile_hook(
            _ntff_profile_via_ctypes("/opt/axon/libaxon_pjrt.so")
        )
        from concourse import bass_utils

        bass_utils.upload_artifacts = lambda tmpdir: tmpdir
    except Exception:
        pass


def _enable_ldw_opt():
    """Flip walrus --enable-ldw-opt to true: consecutive matmuls that reuse
    the same stationary operand then skip the redundant LDWEIGHTS."""
    from concourse import bass_utils

    if getattr(bass_utils, "_ldw_patched", False):
        return
    orig = bass_utils.run_command

    def patched(cmd, **kw):
        cmd = [
            "--enable-ldw-opt=true" if c == "--enable-ldw-opt=false" else c
            for c in cmd
        ]
        return orig(cmd, **kw)

    bass_utils.run_command = patched
    bass_utils._ldw_patched = True


def _chunks(total, size):
    return [(a, min(a + size, total)) for a in range(0, total, size)]


def build_nc(use_collective=False, act_span=2048):
    from concourse import bacc, bass, mybir, tile

    fp32 = mybir.dt.float32
    bf16 = mybir.dt.bfloat16
    i32 = mybir.dt.int32

    nc = bacc.Bacc("TRN2", target_bir_lowering=False, debug=False, num_devices=CORES)
    # Keep matmuls un-split (no standalone InstLdweights) so walrus'
    # ldw-opt can elide redundant weight loads for consecutive matmuls
    # that share the same stationary operand.
    nc.move_matmul_waits_to_ldweights = lambda: None

    # ------------------------------------------------------------ parameters
    def param(name, shape, dt):
        return nc.declare_dram_parameter(name, list(shape), dt, isOutput=False)

    xt_d = param("xt", [KT, 128, RPC], bf16)           # x shard, transposed
    xr_d = param("xr", [TILES, 128, D], bf16)          # x shard, row-major
    gidx_d = param("gidx", [TILES, 128, 1], i32)       # head gather row idx
    r0idx_d = param("r0idx", [TILES, 128, 1], i32)
    r1idx_d = param("r1idx", [TILES, 128, 1], i32)
    valid_d = param("validm", [128, TILES], fp32)      # [p, t] masks
    c0m_d = param("c0m", [128, TILES], fp32)
    c1m_d = param("c1m", [128, TILES], fp32)
    hwt_d = param("hwt", [KT, 128, HEAD], bf16)        # head_w.T k-tiles
    hwr_d = param("hwr", [HEAD, D], bf16)              # head_w row-major
    p0t_d = param("p0t", [KT, 128, H0], bf16)
    p1t_d = param("p1t", [KT, 128, H1], bf16)
    o0t_d = param("o0t", [2, 128, C0], bf16)           # out0.T k-tiles
    o0r_d = param("o0r", [C0, H0], bf16)
    o1t_d = param("o1t", [128, 20 * 1024], bf16)       # out1.T quadrant-packed
    o1r_d = param("o1r", [C1, H1], bf16)

    out_shape = [1, 1] if use_collective else [1, 2]
    out_d = nc.declare_dram_parameter("out", out_shape, fp32, isOutput=True)

    head_slices = _chunks(HEAD, 512)
    c0_chunks = _chunks(C0, 2048)       # 4 chunks
    c1_chunks = _chunks(C1, 2048)       # 20 chunks

    def n_parts(total):
        return sum(len(_chunks(b - a, act_span)) for a, b in _chunks(total, 2048))

    nh, nc0p, nc1p = len(_chunks(HEAD, 512)), n_parts(C0), n_parts(C1)

    with tile.TileContext(nc) as tc:
        with (
            tc.tile_pool(name="res", bufs=1) as res,       # resident SBUF
            tc.tile_pool(name="wstream", bufs=2) as wst,   # streamed weights
            tc.tile_pool(name="scr", bufs=1) as scr,       # ACT exp scratch
            tc.tile_pool(name="dram", bufs=1, space="DRAM") as dram,
        ):
            # -------------------------------------------- resident loads
            # Order matters: it is both the DMA queue order and (roughly) the
            # per-engine program order. xt + head slices first so the head
            # matmul pipeline starts as soon as (k=0, s=0) land.
            xt_sb = []
            for k in range(KT):
                t_ = res.tile([128, RPC], bf16, tag=f"xt{k}", name=f"xt{k}")
                nc.sync.dma_start(out=t_[:, :], in_=xt_d.ap()[k])
                xt_sb.append(t_)
            p0t_sb, p1t_sb = [], []
            for k in range(KT):
                a = res.tile([128, H0], bf16, tag=f"p0t{k}", name=f"p0t{k}")
                nc.sync.dma_start(out=a[:, :], in_=p0t_d.ap()[k])
                p0t_sb.append(a)
                b = res.tile([128, H1], bf16, tag=f"p1t{k}", name=f"p1t{k}")
                nc.sync.dma_start(out=b[:, :], in_=p1t_d.ap()[k])
                p1t_sb.append(b)
            # prefetch the first cluster-1 weight chunks so the c1 pipeline
            # starts before the bulk resident loads finish
            N_W1_PREF = 3
            w1_tiles = {}
            for ci in range(N_W1_PREF):
                w = wst.tile([128, 1024], bf16, tag="w1", name="w1", bufs=N_W1_PREF + 2)
                nc.sync.dma_start(
                    out=w[:, :], in_=o1t_d.ap()[:, 1024 * ci : 1024 * (ci + 1)]
                )
                w1_tiles[ci] = w
            hwt_sb = {}
            for k in range(KT):
                for si, (s0, s1) in enumerate(head_slices):
                    t_ = res.tile(
                        [128, s1 - s0], bf16, tag=f"hwt{k}_{si}", name=f"hwt{k}_{si}"
                    )
                    nc.sync.dma_start(out=t_[:, :], in_=hwt_d.ap()[k, :, s0:s1])
                    hwt_sb[(k, si)] = t_
            xr_sb = []
            for t in range(TILES):
                t_ = res.tile([128, D], bf16, tag=f"xr{t}", name=f"xr{t}")
                nc.sync.dma_start(out=t_[:, :], in_=xr_d.ap()[t])
                xr_sb.append(t_)
            idx_sb = {}
            for name, d in (("g", gidx_d), ("r0", r0idx_d), ("r1", r1idx_d)):
                for t in range(TILES):
                    t_ = res.tile(
                        [128, 1], i32, tag=f"idx{name}{t}", name=f"idx{name}{t}"
                    )
                    nc.sync.dma_start(out=t_[:, :], in_=d.ap()[t])
                    idx_sb[(name, t)] = t_
            valid_sb = res.tile([128, TILES], fp32, tag="valid", name="valid")
            c0m_sb = res.tile([128, TILES], fp32, tag="c0m", name="c0m")
            c1m_sb = res.tile([128, TILES], fp32, tag="c1m", name="c1m")
            nc.sync.dma_start(out=valid_sb[:, :], in_=valid_d.ap()[:, :])
            nc.sync.dma_start(out=c0m_sb[:, :], in_=c0m_d.ap()[:, :])
            nc.sync.dma_start(out=c1m_sb[:, :], in_=c1m_d.ap()[:, :])

            # -------------------------------------------- indirect gathers
            hwg_sb, o0g_sb, o1g_sb = [], [], []
            for t in range(TILES):
                g = res.tile([128, D], bf16, tag=f"hwg{t}", name=f"hwg{t}")
                nc.gpsimd.indirect_dma_start(
                    out=g[:, :], out_offset=None, in_=hwr_d.ap()[:, :],
                    in_offset=bass.IndirectOffsetOnAxis(
                        ap=idx_sb[("g", t)][:, :1], axis=0
                    ),
                )
                hwg_sb.append(g)
                g0 = res.tile([128, H0], bf16, tag=f"o0g{t}", name=f"o0g{t}")
                nc.gpsimd.indirect_dma_start(
                    out=g0[:, :], out_offset=None, in_=o0r_d.ap()[:, :],
                    in_offset=bass.IndirectOffsetOnAxis(
                        ap=idx_sb[("r0", t)][:, :1], axis=0
                    ),
                )
                o0g_sb.append(g0)
                g1 = res.tile([128, H1], bf16, tag=f"o1g{t}", name=f"o1g{t}")
                nc.gpsimd.indirect_dma_start(
                    out=g1[:, :], out_offset=None, in_=o1r_d.ap()[:, :],
                    in_offset=bass.IndirectOffsetOnAxis(
                        ap=idx_sb[("r1", t)][:, :1], axis=0
                    ),
                )
                o1g_sb.append(g1)

            # -------------------------------------------- SBUF result tiles
            p0T_sb = res.tile([128, S_C0], bf16, tag="p0T_a", name="p0T_a")
            p0T_sb2 = res.tile([128, S_C0], bf16, tag="p0T_b", name="p0T_b")
            p1T_sb = res.tile([128, S_C1 + S_PAD], bf16, tag="p1T", name="p1T")
            p0r_sb = {}
            p1r_sb = {}
            for t, m in C0_TILES:
                p0r_sb[t] = res.tile([128, H0], bf16, tag=f"p0r{t}", name=f"p0r{t}")
            for t, m in C1_TILES:
                p1r_sb[t] = res.tile([128, H1], bf16, tag=f"p1r{t}", name=f"p1r{t}")
            hd_sb = res.tile([128, TILES], fp32, tag="hd", name="hd")
            cd0_sb = res.tile([128, TILES], fp32, tag="cd0", name="cd0")
            cd1_sb = res.tile([128, TILES], fp32, tag="cd1", name="cd1")
            F_sb = res.tile([128, 3 * TILES], fp32, tag="F", name="F")
            zh_parts = res.tile([128, TILES * nh], fp32, tag="zhp", name="zhp")
            zc0_parts = res.tile([128, TILES * nc0p], fp32, tag="zc0p", name="zc0p")
            zc1_parts = res.tile([128, TILES * nc1p], fp32, tag="zc1p", name="zc1p")
            exp_scr = scr.tile([128, 2048], bf16, tag="exp", name="exp")

            # sections only write their own tiles/partitions; everything else
            # must be a harmless finite value (exp-sum 1.0 -> Ln finite).
            nc.gpsimd.memset(zc0_parts[:, :], 1.0)
            nc.gpsimd.memset(zc1_parts[:, :], 1.0)
            nc.gpsimd.memset(zh_parts[:, :], 1.0)
            nc.gpsimd.memset(cd0_sb[:, :], 0.0)
            nc.gpsimd.memset(cd1_sb[:, :], 0.0)

            def act_exp(ps, m, cw, parts, t, npart, base):
                """exp + accumulate psum[:m, :cw] in act_span pieces."""
                for j, (a0, a1) in enumerate(_chunks(cw, act_span)):
                    col = t * npart + base + j
                    nc.scalar.activation(
                        exp_scr[:m, a0:a1],
                        ps[:m, a0:a1],
                        mybir.ActivationFunctionType.Exp,
                        accum_out=parts[:m, col : col + 1],
                    )

            with tc.tile_pool(name="big", bufs=2, space="PSUM") as bigp:

                def big_psum():
                    return bigp.tile([128, 2048], fp32, tag="big", name="big")

                # ---------------- projections first (warms PE, unblocks c1)
                # p1T: the c1 section rows (tiles 0..3 incl pad)
                ps = big_psum()
                for k in range(KT):
                    nc.tensor.matmul(
                        ps[:64, : S_C1 + S_PAD],
                        lhsT=p1t_sb[k][:, :],
                        rhs=xt_sb[k][:, : S_C1 + S_PAD],
                        start=(k == 0),
                        stop=(k == KT - 1),
                    )
                nc.vector.tensor_copy(p1T_sb[0:64, :], ps[:64, : S_C1 + S_PAD])
                nc.vector.tensor_copy(p1T_sb[64:128, :], ps[:64, : S_C1 + S_PAD])
                # p0T: only the c0 section rows (tile 4)
                c0_t0 = C0_TILES[0][0]
                for mi, dst in ((0, p0T_sb), (1, p0T_sb2)):
                    ps = big_psum()
                    for k in range(KT):
                        nc.tensor.matmul(
                            ps[:, : S_C0],
                            lhsT=p0t_sb[k][:, 128 * mi : 128 * (mi + 1)],
                            rhs=xt_sb[k][:, 128 * c0_t0 : 128 * c0_t0 + S_C0],
                            start=(k == 0),
                            stop=(k == KT - 1),
                        )
                    nc.vector.tensor_copy(dst[:, :], ps[:, : S_C0])
                # row-major projections for the dot products
                for t, m in C0_TILES:
                    ps = big_psum()
                    for k in range(KT):
                        nc.tensor.matmul(
                            ps[:m, :H0],
                            lhsT=xt_sb[k][:, 128 * t : 128 * t + m],
                            rhs=p0t_sb[k][:, :],
                            start=(k == 0),
                            stop=(k == KT - 1),
                        )
                    nc.vector.tensor_copy(p0r_sb[t][:m, :], ps[:m, :H0])
                for t, m in C1_TILES:
                    ps = big_psum()
                    for k in range(KT):
                        nc.tensor.matmul(
                            ps[:m, :H1],
                            lhsT=xt_sb[k][:, 128 * t : 128 * t + m],
                            rhs=p1t_sb[k][:, :],
                            start=(k == 0),
                            stop=(k == KT - 1),
                        )
                    nc.vector.tensor_copy(p1r_sb[t][:m, :], ps[:m, :H1])

                # ---------------- merged schedule: c1 (chunk, tile) units
                # paced with fine-grained head/c0 fillers. c1 is ACT-heavy /
                # PE-light; head is the reverse. Head fillers are single
                # 512-col slices so no unit holds a PSUM slot for long.
                def emit_head_slice(t, m, si):
                    s0, s1 = head_slices[si]
                    ps = big_psum()
                    for k in range(KT):
                        nc.tensor.matmul(
                            ps[:m, : s1 - s0],
                            lhsT=xt_sb[k][:, 128 * t : 128 * t + m],
                            rhs=hwt_sb[(k, si)][:, :],
                            start=(k == 0),
                            stop=(k == KT - 1),
                        )
                    nc.scalar.activation(
                        exp_scr[:m, : s1 - s0],
                        ps[:m, : s1 - s0],
                        mybir.ActivationFunctionType.Exp,
                        accum_out=zh_parts[:m, t * nh + si : t * nh + si + 1],
                    )

                def emit_c0_chunk(ci):
                    c0, c1_ = c0_chunks[ci]
                    w = [
                        wst.tile([128, 2048], bf16, tag="w0a", name="w0a"),
                        wst.tile([128, 2048], bf16, tag="w0b", name="w0b"),
                    ]
                    cw = c1_ - c0
                    for k in range(2):
                        nc.sync.dma_start(
                            out=w[k][:, :cw], in_=o0t_d.ap()[k, :, c0:c1_]
                        )
                    pbase = sum(
                        len(_chunks(b - a, act_span)) for a, b in c0_chunks[:ci]
                    )
                    for t, m in C0_TILES:
                        ps = big_psum()
                        p0T = (p0T_sb, p0T_sb2)
                        for k in range(2):
                            for s0, s1 in _chunks(cw, 512):
                                nc.tensor.matmul(
                                    ps[:m, s0:s1],
                                    lhsT=p0T[k][:, :m],
                                    rhs=w[k][:, s0:s1],
                                    start=(k == 0),
                                    stop=(k == 1),
                                )
                        act_exp(ps, m, cw, zc0_parts, t, nc0p, pbase)

                fillers = [
                    ("head", t, si)
                    for t, m in HEAD_TILES
                    for si in range(len(head_slices))
                ] + [("c0", ci, 0) for ci in range(len(c0_chunks))]
                n_units = len(c1_chunks) * len(C1_TILES)
                fper = len(fillers) / n_units
                fcredit = 0.0
                fi = 0
                for ci in range(len(c1_chunks)):
                    c0, c1_ = c1_chunks[ci]
                    cw = c1_ - c0          # real classes this chunk (<= 2048)
                    if ci in w1_tiles:
                        w = w1_tiles.pop(ci)
                    else:
                        w = wst.tile(
                            [128, 1024], bf16, tag="w1", name="w1", bufs=N_W1_PREF + 2
                        )
                        nc.sync.dma_start(
                            out=w[:, :], in_=o1t_d.ap()[:, 1024 * ci : 1024 * (ci + 1)]
                        )
                    pbase = sum(
                        len(_chunks(b - a, act_span)) for a, b in c1_chunks[:ci]
                    )
                    for t, m in C1_TILES:
                        ps = big_psum()
                        # quadrant-packed: top half of w = classes [0,1024),
                        # bottom half = classes [1024,2048) of this chunk.
                        # Alternate quadrants so LDWEIGHTS overlaps matmuls.
                        mms_a = [(0, s0, s1) for s0, s1 in _chunks(min(cw, 1024), 512)]
                        mms_b = (
                            [(1, s0, s1) for s0, s1 in _chunks(cw - 1024, 512)]
                            if cw > 1024
                            else []
                        )
                        order = []
                        for i in range(max(len(mms_a), len(mms_b))):
                            if i < len(mms_a):
                                order.append(mms_a[i])
                            if i < len(mms_b):
                                order.append(mms_b[i])
                        for h, s0, s1 in order:
                            nc.tensor.matmul(
                                ps[:m, 1024 * h + s0 : 1024 * h + s1],
                                lhsT=p1T_sb[64 * h : 64 * h + 64, 128 * t : 128 * t + m],
                                rhs=w[64 * h : 64 * h + 64, s0:s1],
                                start=True,
                                stop=True,
                            )
                        act_exp(ps, m, cw, zc1_parts, t, nc1p, pbase)
                        fcredit += fper
                        while fcredit >= 1.0 and fi < len(fillers):
                            kind, a, b = fillers[fi]
                            fi += 1
                            fcredit -= 1.0
                            if kind == "head":
                                emit_head_slice(a, 128, b)
                            else:
                                emit_c0_chunk(a)
                while fi < len(fillers):
                    kind, a, b = fillers[fi]
                    fi += 1
                    if kind == "head":
                        emit_head_slice(a, 128, b)
                    else:
                        emit_c0_chunk(a)

                # ---------------- dots (DVE, off critical path)
                dots_scr = scr.tile([128, D], bf16, tag="dots", name="dots")
                for t in range(TILES):
                    nc.vector.tensor_tensor(
                        dots_scr[:, :D], xr_sb[t][:, :], hwg_sb[t][:, :],
                        mybir.AluOpType.mult,
                    )
                    nc.vector.tensor_reduce(
                        out=hd_sb[:, t : t + 1], in_=dots_scr[:, :D],
                        axis=mybir.AxisListType.X, op=mybir.AluOpType.add,
                    )
                for t, m in C0_TILES:
                    nc.vector.tensor_tensor(
                        dots_scr[:m, :H0], p0r_sb[t][:m, :], o0g_sb[t][:m, :],
                        mybir.AluOpType.mult,
                    )
                    nc.vector.tensor_reduce(
                        out=cd0_sb[:m, t : t + 1], in_=dots_scr[:m, :H0],
                        axis=mybir.AxisListType.X, op=mybir.AluOpType.add,
                    )
                for t, m in C1_TILES:
                    nc.vector.tensor_tensor(
                        dots_scr[:m, :H1], p1r_sb[t][:m, :], o1g_sb[t][:m, :],
                        mybir.AluOpType.mult,
                    )
                    nc.vector.tensor_reduce(
                        out=cd1_sb[:m, t : t + 1], in_=dots_scr[:m, :H1],
                        axis=mybir.AxisListType.X, op=mybir.AluOpType.add,
                    )

            # -------------------------------------------- per-row loss
            for t in range(TILES):
                for sec_i, (parts, npart) in enumerate(
                    ((zh_parts, nh), (zc0_parts, nc0p), (zc1_parts, nc1p))
                ):
                    nc.vector.tensor_reduce(
                        out=F_sb[:, sec_i * TILES + t : sec_i * TILES + t + 1],
                        in_=parts[:, t * npart : (t + 1) * npart],
                        axis=mybir.AxisListType.X,
                        op=mybir.AluOpType.add,
                    )
            L_sb = res.tile([128, 3 * TILES], fp32, tag="L", name="L")
            nc.scalar.activation(
                L_sb[:, :], F_sb[:, :], mybir.ActivationFunctionType.Ln
            )
            T1 = res.tile([128, TILES], fp32, tag="T1", name="T1")
            T2 = res.tile([128, TILES], fp32, tag="T2", name="T2")
            R_sb = res.tile([128, 2 * TILES], fp32, tag="R", name="R")
            sub, mult, add = (
                mybir.AluOpType.subtract,
                mybir.AluOpType.mult,
                mybir.AluOpType.add,
            )
            nc.vector.tensor_tensor(T1[:, :], hd_sb[:, :], L_sb[:, 0:TILES], sub)
            nc.vector.tensor_tensor(
                T2[:, :], cd0_sb[:, :], L_sb[:, TILES : 2 * TILES], sub
            )
            nc.vector.tensor_tensor(T2[:, :], T2[:, :], c0m_sb[:, :], mult)
            nc.vector.tensor_tensor(T1[:, :], T1[:, :], T2[:, :], add)
            nc.vector.tensor_tensor(
                T2[:, :], cd1_sb[:, :], L_sb[:, 2 * TILES : 3 * TILES], sub
            )
            nc.vector.tensor_tensor(T2[:, :], T2[:, :], c1m_sb[:, :], mult)
            nc.vector.tensor_tensor(T1[:, :], T1[:, :], T2[:, :], add)
            nc.vector.tensor_tensor(R_sb[:, 0:TILES], T1[:, :], valid_sb[:, :], mult)
            nc.vector.tensor_copy(R_sb[:, TILES : 2 * TILES], valid_sb[:, :])

            # -------------------------------------------- reduce across rows
            ones_sb = res.tile([128, 1], fp32, tag="ones", name="ones")
            nc.gpsimd.memset(ones_sb[:, :], 1.0)
            S_sb = res.tile([1, 2 * TILES], fp32, tag="S", name="S")
            cc_in = res.tile([1, 16], fp32, tag="ccin", name="ccin")
            nc.gpsimd.memset(cc_in[:, :], 0.0)
            with tc.tile_pool(name="sm", bufs=1, space="PSUM") as smp:
                ps = smp.tile([1, 2 * TILES], fp32, tag="sm", name="sm")
                nc.tensor.matmul(
                    ps[:, :], lhsT=ones_sb[:, :], rhs=R_sb[:, :], start=True, stop=True
                )
                nc.vector.tensor_copy(S_sb[:, :], ps[:, :])
            nc.vector.tensor_reduce(
                out=cc_in[0:1, 0:1],
                in_=S_sb[0:1, 0:TILES],
                axis=mybir.AxisListType.X,
                op=add,
            )
            nc.vector.tensor_reduce(
                out=cc_in[0:1, 1:2],
                in_=S_sb[0:1, TILES : 2 * TILES],
                axis=mybir.AxisListType.X,
                op=add,
            )

            if use_collective:
                cc_in_d = dram.tile([1, 16], fp32, name="ccind")
                cc_out_d = dram.tile([1, 16], fp32, name="ccoutd")
                nc.gpsimd.dma_start(out=cc_in_d[:, :], in_=cc_in[:, :])
                nc.gpsimd.collective_compute(
                    "AllReduce",
                    mybir.AluOpType.add,
                    replica_groups=[list(range(CORES))],
                    ins=[cc_in_d.opt()],
                    outs=[cc_out_d.opt()],
                )
                AR = res.tile([1, 16], fp32, tag="AR", name="AR")
                nc.gpsimd.dma_start(out=AR[:, :], in_=cc_out_d[:, :])
                rec = res.tile([1, 1], fp32, tag="rec", name="rec")
                nc.vector.reciprocal(rec[:, :], AR[0:1, 1:2])
                lossv = res.tile([1, 1], fp32, tag="loss", name="loss")
                nc.vector.tensor_tensor(lossv[:, :], AR[0:1, 0:1], rec[:, :], mult)
                nc.vector.tensor_scalar_mul(lossv[:, :], lossv[:, :], -1.0)
                nc.sync.dma_start(out=out_d.ap()[:, :], in_=lossv[:, :])
            else:
                nc.sync.dma_start(out=out_d.ap()[:, :], in_=cc_in[0:1, 0:2])

    nc.compile()
    return nc


def _pack_o1t(out1):
    """out1.T quadrant-packed: per 2048-class chunk, classes [0,1024) go to
    partitions 0:64, classes [1024,2048) to partitions 64:128."""
    o1t = np.zeros((H1, 20 * 2048), dtype=np.float32)
    o1t[:, :C1] = out1.T
    o1t = o1t.reshape(H1, 20, 2, 1024).transpose(2, 0, 1, 3).reshape(128, 20 * 1024)
    return np.ascontiguousarray(o1t).astype(BF16)


def host_prep(x, target, head_w, proj0, out0, proj1, out1):
    """Class-sorted sharding + layout/dtype prep. Returns per-core in_maps."""
    x = np.asarray(x, dtype=np.float32)
    target = np.asarray(target).astype(np.int64)
    head_w = np.asarray(head_w, dtype=np.float32)
    proj0 = np.asarray(proj0, dtype=np.float32)
    out0 = np.asarray(out0, dtype=np.float32)
    proj1 = np.asarray(proj1, dtype=np.float32)
    out1 = np.asarray(out1, dtype=np.float32)

    valid = target != IGNORE
    tt = np.where(valid, target, 0)
    is_c0 = valid & (tt >= C0_LOW) & (tt < C0_HIGH)
    is_c1 = valid & (tt >= C0_HIGH)
    g = np.where(is_c1, 2001, np.where(is_c0, 2000, np.clip(tt, 0, C0_LOW - 1)))
    r0 = np.clip(tt - C0_LOW, 0, C0 - 1)
    r1 = np.clip(tt - C0_HIGH, 0, C1 - 1)

    idx_c1 = np.where(is_c1)[0]
    idx_c0 = np.where(is_c0)[0]
    idx_hd = np.where(~(is_c0 | is_c1))[0]
    assert len(idx_c1) <= CAP_C1, f"c1 overflow: {len(idx_c1)} > {CAP_C1}"
    assert len(idx_c0) <= CAP_C0, f"c0 overflow: {len(idx_c0)} > {CAP_C0}"
    assert len(idx_hd) <= CAP_HD, f"head overflow: {len(idx_hd)} > {CAP_HD}"

    def slots(idx, cap):
        a = np.full(cap, -1, dtype=np.int64)
        a[: len(idx)] = idx
        return a.reshape(CORES, cap // CORES)

    s1 = slots(idx_c1, CAP_C1)
    s0 = slots(idx_c0, CAP_C0)
    sh = slots(idx_hd, CAP_HD)
    pad = np.full((CORES, S_PAD), -1, dtype=np.int64)
    rows_per_core = np.concatenate([s1, pad, s0, sh], axis=1)  # [CORES, RPC]

    w_common = {
        "hwt": np.ascontiguousarray(head_w.T).astype(BF16).reshape(KT, 128, HEAD),
        "hwr": head_w.astype(BF16),
        "p0t": np.ascontiguousarray(proj0.T).astype(BF16).reshape(KT, 128, H0),
        "p1t": np.ascontiguousarray(proj1.T).astype(BF16).reshape(KT, 128, H1),
        "o0t": np.ascontiguousarray(out0.T).astype(BF16).reshape(2, 128, C0),
        "o0r": out0.astype(BF16),
        "o1t": _pack_o1t(out1),
        "o1r": out1.astype(BF16),
    }

    in_maps = []
    for c in range(CORES):
        rows = rows_per_core[c]
        real = rows >= 0
        rr = np.where(real, rows, 0)
        xs = x[rr] * real[:, None]
        m = dict(w_common)
        m["xt"] = np.ascontiguousarray(xs.T).astype(BF16).reshape(KT, 128, RPC)
        m["xr"] = xs.astype(BF16).reshape(TILES, 128, D)
        m["gidx"] = (g[rr] * real).astype(np.int32).reshape(TILES, 128, 1)
        m["r0idx"] = (r0[rr] * real).astype(np.int32).reshape(TILES, 128, 1)
        m["r1idx"] = (r1[rr] * real).astype(np.int32).reshape(TILES, 128, 1)

        def pt(v):
            return np.ascontiguousarray(v.astype(np.float32).reshape(TILES, 128).T)

        m["validm"] = pt(valid[rr] & real)
        m["c0m"] = pt(is_c0[rr] & real)
        m["c1m"] = pt(is_c1[rr] & real)
        in_maps.append(m)
    return in_maps


def kernel(x, target, head_w, proj0, out0, proj1, out1):
    _install_profile_hook()
    from concourse.bass_utils import run_bass_kernel_spmd

    use_collective = False
    key = ("nc", use_collective)
    if key not in _CACHE:
        _CACHE[key] = build_nc(use_collective)
    nc = _CACHE[key]

    in_maps = host_prep(x, target, head_w, proj0, out0, proj1, out1)
    res = run_bass_kernel_spmd(nc, in_maps, core_ids=list(range(CORES)), trace=False)
    if use_collective:
        loss = np.float32(res.results[0]["out"][0, 0])
    else:
        acc = np.zeros(2, dtype=np.float64)
        for r in res.results:
            acc += r["out"][0].astype(np.float64)
        loss = np.float32(-acc[0] / acc[1])
    return np.asarray(loss, dtype=np.float32)


if __name__ == "__main__":
    rng = np.random.default_rng(0)
    x = rng.standard_normal((N_ROWS, D), dtype=np.float32)
    target = rng.integers(0, NCLS, size=(N_ROWS,)).astype(np.int64)
    target[rng.random(N_ROWS) < 0.1] = IGNORE
    s = 1.0 / np.sqrt(D)
    head_w = rng.standard_normal((HEAD, D), dtype=np.float32) * s
    proj0 = rng.standard_normal((H0, D), dtype=np.float32) * s
    out0 = rng.standard_normal((C0, H0), dtype=np.float32) / np.sqrt(H0)
    proj1 = rng.standard_normal((H1, D), dtype=np.float32) * s
    out1 = rng.standard_normal((C1, H1), dtype=np.float32) / np.sqrt(H1)
    print(kernel(x=x, target=target, head_w=head_w, proj0=proj0, out0=out0,
                 proj1=proj1, out1=out1))
